# revision 27
# baseline (speedup 1.0000x reference)
"""Elastic 2D velocity-stress FD (4th order, CPML) on 8 trn2 NeuronCores.

Sharding: 8 cores = 2 shots x 4 y-slabs (sizes [88,60,60,88]) with redundant
halos (each core owns a 128-row window of the 296-row padded grid; >=34-row
halos make the 64-step simulation exact to ~3e-9 with ZERO inter-core
communication — validated empirically against the reference).

Per-core layout: y on partitions (128), x on free dim (300 = 2 pad + 296 + 2 pad).
 - y-derivatives, CPML-y recursions, and all constant-coefficient linear
   combinations run on the TensorEngine as banded/diagonal matmuls accumulating
   into PSUM.
 - x-derivatives are 4 tap-matmuls (scaled identity x shifted-window rhs).
 - Only 2D-coefficient pointwise multiplies + CPML-x strip recursions run on
   VectorE; PSUM->SBUF copybacks on ScalarE.
 - Receivers are extracted on-chip each step (row-select matmul into PSUM +
   one DVE multiply-reduce against a column one-hot), so the only output is
   a [64, NT] trace block per core instead of the full wavefield.
Host does all per-core specialization (band matrices, coefficient fields,
source outer-product factors, receiver selectors); the final gather is a sum
of the 4 slab cores per shot.

The jit-wrapped executable is built and compiled once at import; the axon
tunnel is warmed with tiny transfers first (the first bulk transfer on a
cold tunnel runs ~150x slower than subsequent ones).
"""
import numpy as np

# --- problem constants (hardcoded per spec) ---
NY_I = NX_I = 256
PML = 20
DX = 4.0
DT = 5e-4
NT = 64
C1, C2 = 9.0 / 8.0, -1.0 / 24.0
NYP = NY_I + 2 * PML      # 296
NXP = NX_I + 2 * PML      # 296
W = NXP + 4               # 300 padded width; data cols 2..297
P = 128                   # partitions per core window
G0 = [0, 54, 114, 168]    # per-slab window start row (global padded coords)
SLABS = [(0, 88), (88, 148), (148, 208), (208, 296)]  # owned rows
NSRC = 8
NREC = 64
N_SHOT = 2
# x-stencil taps: d[x] = sum_k c_k * f[x+delta_k]
TAPC = [C1 / DX, -C1 / DX, C2 / DX, -C2 / DX]
DBWD = [0, -1, 1, -2]
DFWD = [1, 0, 2, -1]
# strip (x-PML) columns in padded coords: [2,22) and [278,298)
STRIP0 = [2, 278]
SW = 20

# packed const layout (columns of the single "cst" input).
# note DT*(l2m-lamb)/2 == DT*mu, so ab2's B slot doubles as dtmu.
C_AB = 0                       # A/B coeff pair (2 x 300); B == dt*mu
C_DTB = C_AB + 2 * W           # dt*buoy        (300)
C_BXS = C_DTB + W              # x-strip coeffs (2 sides x 20)
C_BY = C_BXS + 2 * SW
C_AY = C_BY + 1
C_RROW = C_AY + 1              # receiver row selector transposed (128 x 64)
CTOT = C_RROW + NREC

_cache = {}


def _host_prep(lamb, mu, buoyancy):
    f32 = np.float32
    lambp = np.pad(lamb.astype(f32), PML, mode='edge')
    mup = np.pad(mu.astype(f32), PML, mode='edge')
    buoyp = np.pad(buoyancy.astype(f32), PML, mode='edge')
    l2m = lambp + 2.0 * mup
    max_vel = np.max(np.sqrt(l2m * buoyp)).astype(f32)
    sig_max = f32(3.0 * max_vel * np.log(f32(1000.0)) / (2.0 * PML * DX))

    def prof(n):
        i = np.arange(n, dtype=f32)
        d = np.maximum(np.clip(PML - i, 0.0, None),
                       np.clip(i - (n - 1 - PML), 0.0, None)) / PML
        return sig_max * d * d

    by = np.exp(-prof(NYP) * f32(DT)).astype(f32)   # [296]
    bx = np.exp(-prof(NXP) * f32(DT)).astype(f32)   # [296]
    return lambp, mup, buoyp, l2m, by, bx


def _band(fwd):
    """Local [128,128] band matrix M with out = M @ f (rows=local out row)."""
    B = np.zeros((P, P), np.float32)
    taps = zip(DFWD if fwd else DBWD, TAPC)
    for off, c in taps:
        for m in range(P):
            k = m + off
            if 0 <= k < P:
                B[m, k] += c
    return B


def _core_inputs(core, lambp, mup, buoyp, l2m, by, bx, amps, src_loc, rec_loc,
                 nsteps, t0):
    """Build the ExternalInput dict for one core."""
    f32 = np.float32
    s, j = divmod(core, 4)
    g0 = G0[j]
    rs = slice(g0, g0 + P)
    byl = by[rs]
    ayl = byl - 1.0

    def widen(a):  # [128,296] -> [128,300] with zero pads
        out = np.zeros((P, W), f32)
        out[:, 2:2 + NXP] = a
        return out

    dtbuoy = widen(f32(DT) * buoyp[rs])
    A = widen(f32(DT) * (l2m[rs] + lambp[rs]) * 0.5)
    Bc = widen(f32(DT) * mup[rs])   # == DT*(l2m-lamb)/2 == dt*mu
    ab2 = np.stack([A, Bc], 1)      # [128,2,300]
    bxs = np.zeros((P, 2, SW), f32)
    for side, c0 in enumerate(STRIP0):
        bxs[:, side, :] = bx[c0 - 2:c0 - 2 + SW][None, :]

    srcamp = np.ascontiguousarray(amps[s, :, t0:t0 + nsteps]).astype(f32)
    srcrow = np.zeros((NSRC, P), f32)
    srcr = np.zeros((NSRC, W), f32)
    for i in range(NSRC):
        y = int(src_loc[s, i, 0]) + PML
        x = int(src_loc[s, i, 1]) + PML
        srcr[i, 2 + x] = 1.0
        if g0 <= y < g0 + P:
            srcrow[i, y - g0] = 1.0

    lo, hi = SLABS[j]
    rrowT = np.zeros((P, NREC), f32)
    rcol = np.zeros((NREC, W), f32)
    for r in range(NREC):
        y = int(rec_loc[s, r, 0]) + PML
        x = int(rec_loc[s, r, 1]) + PML
        if lo <= y < hi:
            rrowT[y - g0, r] = 1.0
            rcol[r, 2 + x] = 1.0

    cst = np.zeros((P, CTOT), f32)
    cst[:, C_AB:C_AB + 2 * W] = ab2.reshape(P, 2 * W)
    cst[:, C_DTB:C_DTB + W] = dtbuoy
    cst[:, C_BXS:C_BXS + 2 * SW] = bxs.reshape(P, 2 * SW)
    cst[:, C_BY] = byl
    cst[:, C_AY] = ayl
    cst[:, C_RROW:C_RROW + NREC] = rrowT
    return {"cst": cst, "srcamp": srcamp, "srcrow": srcrow,
            "srcr": srcr, "rcol": rcol}


def _wts_np():
    """Constant band/tap weight slots [128, 6*128] (input-independent)."""
    f32 = np.float32
    eye = np.eye(P, dtype=f32)
    wts = np.zeros((P, 6, P), f32)
    wts[:, 0] = _band(fwd=False).T
    wts[:, 1] = _band(fwd=True).T
    for k in range(4):
        wts[:, 2 + k] = TAPC[k] * eye
    return wts.reshape(P, 6 * P)


def build_nc(nsteps=NT):
    import concourse.bacc as bacc
    import concourse.tile as tile
    from concourse import mybir

    f32 = mybir.dt.float32

    nc = bacc.Bacc("TRN2", target_bir_lowering=False, debug=False, num_devices=8)
    cst_d = nc.dram_tensor("cst", [P, CTOT], f32, kind="ExternalInput")
    wts_d = nc.dram_tensor("wts", [P, 6 * P], f32, kind="ExternalInput")
    srcamp_d = nc.dram_tensor("srcamp", [NSRC, nsteps], f32, kind="ExternalInput")
    srcrow_d = nc.dram_tensor("srcrow", [NSRC, P], f32, kind="ExternalInput")
    srcr_d = nc.dram_tensor("srcr", [NSRC, W], f32, kind="ExternalInput")
    rcol_d = nc.dram_tensor("rcol", [NREC, W], f32, kind="ExternalInput")
    rec_d = nc.dram_tensor("rec", [NREC, nsteps], f32, kind="ExternalOutput")

    with tile.TileContext(nc) as tc:
        with (
            tc.tile_pool(name="const", bufs=1) as cp,
            tc.tile_pool(name="state", bufs=1) as sp,
            tc.tile_pool(name="scr", bufs=2) as scr,
            tc.tile_pool(name="ps", bufs=1, space="PSUM") as pp,
        ):
            cst = cp.tile([P, CTOT], f32)
            nc.sync.dma_start(cst[:], cst_d[:])
            srcamp_t = cp.tile([NSRC, nsteps], f32)
            nc.sync.dma_start(srcamp_t[:], srcamp_d[:])
            srcrow_t = cp.tile([NSRC, P], f32)
            nc.sync.dma_start(srcrow_t[:], srcrow_d[:])
            srcr_t = cp.tile([NSRC, W], f32)
            nc.sync.dma_start(srcr_t[:], srcr_d[:])
            srcr = srcr_t[:]
            rcol_t = cp.tile([P, W], f32)
            nc.sync.dma_start(rcol_t[0:NREC, :], rcol_d[:])
            rcol = rcol_t[0:NREC, :]
            wtsr = cp.tile([P, 6 * P], f32)
            nc.sync.dma_start(wtsr[:], wts_d[:])
            # weights must be DVE-written so matmuls carry a single wait
            wts = cp.tile([P, 6, P], f32)
            nc.vector.tensor_copy(
                wts[:], wtsr[:].rearrange("p (a b) -> p a b", a=6))
            ab2 = cst[:, C_AB:C_AB + 2 * W].rearrange("p (a b) -> p a b", a=2)
            dtbuoy = cst[:, C_DTB:C_DTB + W]
            dtmu = ab2[:, 1, :]
            bxs = cst[:, C_BXS:C_BXS + 2 * SW].rearrange("p (a b) -> p a b", a=2)
            by_ap = cst[:, C_BY:C_BY + 1]
            ay_ap = cst[:, C_AY:C_AY + 1]
            rrowT = cst[:, C_RROW:C_RROW + NREC]

            v2 = sp.tile([P, 2, W], f32)      # vy | vx
            s2 = sp.tile([P, 2, W], f32)      # syy | sxx
            sxy = sp.tile([P, W], f32)
            my_vel = sp.tile([P, 2, W], f32)  # msyyy | msxyy
            my_str = sp.tile([P, 2, W], f32)  # mvyy | mvxy
            mw_vel = sp.tile([P, 2, W], f32)  # msxyx | msxxx (zero outside strips)
            mw_str = sp.tile([P, 2, W], f32)  # mvxx | mvyx
            rec_full = sp.tile([P, nsteps], f32)
            rec_buf = rec_full[0:NREC, :]
            for t_ in (v2, s2, sxy, my_vel, my_str, mw_vel, mw_str, rec_full):
                nc.vector.memset(t_[:], 0.0)

            ps_ab = pp.tile([P, 2, 512], f32)   # x-stencil taps: d_x pair
            ps_dy = pp.tile([P, 2, 512], f32)   # plain y-band derivs pair (+src)
            ps_st = pp.tile([P, 2, 512], f32)   # stress x-stencil taps pair
            ps_rec_full = pp.tile([P, 512], f32)  # receiver row-gather
            ps_rec = ps_rec_full[0:NREC, :]

            MM = nc.tensor.matmul
            Wt = lambda i: wts[:, i, :]
            vy, vx = v2[:, 0, :], v2[:, 1, :]

            def strips4v(ap2):
                """[P,20] per-var view at left strip -> [P,2,20] both strips."""
                a = ap2.copy()
                a.ap.insert(1, [STRIP0[1] - STRIP0[0], 2])
                return a

            def strip_chain_v(mw, f_, ps_pair):
                """Per-var CPML-x strip recursion (3 DVE ops, FD=40)."""
                d_ = strips4v(ps_pair[:, f_, STRIP0[0]:STRIP0[0] + SW])
                mwv = strips4v(mw[:, f_, STRIP0[0]:STRIP0[0] + SW])
                s_ = scr.tile([P, 2, SW], f32, tag="strip_s")
                nc.vector.tensor_add(s_[:], mwv, d_)
                nc.vector.tensor_mul(s_[:], s_[:], bxs[:])
                nc.vector.tensor_sub(mwv, s_[:], d_)

            Copy = mybir.ActivationFunctionType.Copy
            # expand source injection weights on-chip: z[i,t,:] = amp[i,t]*rowhot[i,:]
            zsrc_t = cp.tile([NSRC, nsteps, P], f32)
            for tt in range(nsteps):
                nc.scalar.activation(zsrc_t[:, tt, :], srcrow_t[:], Copy,
                                     scale=srcamp_t[:, tt:tt + 1])
            srcw = zsrc_t[:]
            for t in range(nsteps):
                sgc = dict(skip_group_check=True)
                # ================= VELOCITY =================
                # PE order: vy's inputs first (B@syy + src), so the vy chain
                # starts while PE still runs sxx taps.
                MM(ps_dy[:, 0, 2:298], Wt(0), s2[:, 0, 2:298], start=True, stop=False, **sgc)
                MM(ps_dy[:, 0, 2:298], srcw[:, t, :], srcr[:, 2:298],
                   start=False, stop=True, **sgc)
                for k in range(4):
                    d = DBWD[k]
                    MM(ps_ab[:, 0, 2:298], Wt(2 + k), sxy[:, 2 + d:298 + d],
                       start=(k == 0), stop=(k == 3), **sgc)
                MM(ps_dy[:, 1, 2:298], Wt(0), sxy[:, 2:298], start=True, stop=True, **sgc)
                # sxx x-derivative on DVE (PE tap block shrinks by 4 MMs):
                # tx = C1'*(f[x]-f[x-1]) + C2'*(f[x+1]-f[x-2]), real units
                tx = scr.tile([P, 296], f32, tag="tx")
                tt1 = scr.tile([P, 296], f32, tag="tt1")
                nc.vector.tensor_sub(tt1[:], s2[:, 1, 2:298], s2[:, 1, 1:297])
                nc.vector.tensor_sub(tx[:], s2[:, 1, 3:299], s2[:, 1, 0:296])
                nc.vector.scalar_tensor_tensor(
                    tx[:], tx[:], C2 / C1, tt1[:],
                    op0=mybir.AluOpType.mult, op1=mybir.AluOpType.add)
                nc.vector.tensor_scalar_mul(tx[:], tx[:], TAPC[0])
                # --- vy chain (DVE, reads PSUM directly) ---
                uy = scr.tile([P, 2, 296], f32, tag="uy")
                g0 = scr.tile([P, 296], f32, tag="g0")
                nc.scalar.activation(g0[:], my_vel[:, 0, 2:298], Copy, scale=by_ap)
                nc.scalar.activation(uy[:, 0, :], ps_dy[:, 0, 2:298], Copy, scale=ay_ap)
                nc.gpsimd.tensor_add(my_vel[:, 0, 2:298], g0[:], uy[:, 0, :])
                strip_chain_v(mw_vel, 0, ps_ab)
                # tree-parallel assembly: a1 = d_y+m' (DVE) || a2 = d_x+mw (ACT+Pool)
                S = scr.tile([P, 2, 296], f32, tag="S")
                wv = scr.tile([P, 2, 296], f32, tag="wv")
                e_ab0 = scr.tile([P, 296], f32, tag="e_ab0")
                a2 = scr.tile([P, 296], f32, tag="a2")
                nc.scalar.copy(e_ab0[:], ps_ab[:, 0, 2:298])
                nc.gpsimd.tensor_add(a2[:], e_ab0[:], mw_vel[:, 0, 2:298])
                nc.vector.tensor_add(S[:, 0, :], ps_dy[:, 0, 2:298], my_vel[:, 0, 2:298])
                nc.vector.tensor_add(S[:, 0, :], S[:, 0, :], a2[:])
                nc.vector.tensor_mul(wv[:, 0, :], dtbuoy[:, 2:298], S[:, 0, :])
                nc.vector.tensor_add(v2[:, 0, 2:298], v2[:, 0, 2:298], wv[:, 0, :])
                # --- receiver extraction: rec[r,t] = sum_xy Rrow[r,y] vy[y,x] Rcol[r,x]
                # (tensor_tensor_reduce wedges the TRN2 exec unit here; use
                #  separate multiply + reduce instead)
                MM(ps_rec[:, 0:W], rrowT, vy, start=True, stop=True, **sgc)
                rprod = scr.tile([P, W], f32, tag="rprod")
                nc.vector.tensor_mul(rprod[0:NREC, :], ps_rec[:, 0:W], rcol)
                nc.vector.reduce_sum(rec_buf[:, t:t + 1], rprod[0:NREC, :],
                                     axis=mybir.AxisListType.X)
                # --- vx chain (ACT drains PSUM, Pool arithmetic) ---
                nc.scalar.activation(uy[:, 1, :], ps_dy[:, 1, 2:298], Copy, scale=ay_ap)
                nc.vector.scalar_tensor_tensor(
                    my_vel[:, 1, 2:298], my_vel[:, 1, 2:298], by_ap, uy[:, 1, :],
                    op0=mybir.AluOpType.mult, op1=mybir.AluOpType.add)
                # var1 strip recursion off the SBUF-resident tx
                d1_ = strips4v(tx[:, 0:SW])
                mwv1 = strips4v(mw_vel[:, 1, STRIP0[0]:STRIP0[0] + SW])
                s1_ = scr.tile([P, 2, SW], f32, tag="strip_s")
                nc.vector.tensor_add(s1_[:], mwv1, d1_)
                nc.vector.tensor_mul(s1_[:], s1_[:], bxs[:])
                nc.vector.tensor_sub(mwv1, s1_[:], d1_)
                e_dy = scr.tile([P, 296], f32, tag="e_dy")
                nc.scalar.copy(e_dy[:], ps_dy[:, 1, 2:298])
                nc.gpsimd.tensor_add(S[:, 1, :], e_dy[:], my_vel[:, 1, 2:298])
                nc.gpsimd.tensor_add(S[:, 1, :], tx[:], S[:, 1, :])
                nc.gpsimd.tensor_add(S[:, 1, 0:296], S[:, 1, 0:296], mw_vel[:, 1, 2:298])
                nc.gpsimd.tensor_mul(wv[:, 1, :], dtbuoy[:, 2:298], S[:, 1, :])
                nc.gpsimd.tensor_add(v2[:, 1, 2:298], v2[:, 1, 2:298], wv[:, 1, :])

                # ================= STRESS =================
                # PE order: vy consumers first (vy finished first).
                MM(ps_dy[:, 0, 2:298], Wt(1), vy[:, 2:298], start=True, stop=True, **sgc)
                for k in range(4):
                    d = DFWD[k]
                    MM(ps_st[:, 1, 2:298], Wt(2 + k), vy[:, 2 + d:298 + d],
                       start=(k == 0), stop=(k == 3), **sgc)
                MM(ps_dy[:, 1, 2:298], Wt(1), vx[:, 2:298], start=True, stop=True, **sgc)
                for k in range(4):
                    d = DFWD[k]
                    MM(ps_st[:, 0, 2:298], Wt(2 + k), vx[:, 2 + d:298 + d],
                       start=(k == 0), stop=(k == 3), **sgc)
                uy2 = scr.tile([P, 2, 296], f32, tag="uy")
                # --- sxy chain (finish first: next velocity needs sxy) ---
                g1 = scr.tile([P, 296], f32, tag="g0")
                nc.scalar.activation(g1[:], my_str[:, 1, 2:298], Copy, scale=by_ap)
                nc.scalar.activation(uy2[:, 1, :], ps_dy[:, 1, 2:298], Copy, scale=ay_ap)
                nc.gpsimd.tensor_add(my_str[:, 1, 2:298], g1[:], uy2[:, 1, :])
                strip_chain_v(mw_str, 1, ps_st)
                T2 = scr.tile([P, 2, 296], f32, tag="T2")
                X2 = scr.tile([P, 2, 296], f32, tag="X2")
                e_t = scr.tile([P, 296], f32, tag="e_t")
                nc.scalar.copy(e_t[:], ps_dy[:, 1, 2:298])
                nc.gpsimd.tensor_add(T2[:, 1, :], e_t[:], my_str[:, 1, 2:298])
                nc.vector.tensor_add(X2[:, 1, :], ps_st[:, 1, 2:298], mw_str[:, 1, 2:298])
                t5 = scr.tile([P, 296], f32, tag="t5")
                nc.gpsimd.tensor_add(t5[:], T2[:, 1, :], X2[:, 1, :])
                nc.gpsimd.tensor_mul(t5[:], dtmu[:, 2:298], t5[:])
                nc.gpsimd.tensor_add(sxy[:, 2:298], sxy[:, 2:298], t5[:])
                # --- syy/sxx chain; sxx finishes before syy (taps need sxx) ---
                nc.scalar.activation(uy2[:, 0, :], ps_dy[:, 0, 2:298], Copy, scale=ay_ap)
                nc.vector.scalar_tensor_tensor(
                    my_str[:, 0, 2:298], my_str[:, 0, 2:298], by_ap, uy2[:, 0, :],
                    op0=mybir.AluOpType.mult, op1=mybir.AluOpType.add)
                strip_chain_v(mw_str, 0, ps_st)
                nc.vector.tensor_add(T2[:, 0, :], ps_dy[:, 0, 2:298], my_str[:, 0, 2:298])
                nc.vector.tensor_add(X2[:, 0, :], ps_st[:, 0, 2:298], mw_str[:, 0, 2:298])
                tpm = scr.tile([P, 2, 296], f32, tag="tpm")
                nc.vector.tensor_add(tpm[:, 0, :], T2[:, 0, :], X2[:, 0, :])
                nc.gpsimd.tensor_sub(tpm[:, 1, :], T2[:, 0, :], X2[:, 0, :])
                c12v = scr.tile([P, 2, 296], f32, tag="c12v")
                nc.vector.tensor_mul(c12v[:], ab2[:, :, 2:298], tpm[:])
                u12 = scr.tile([P, 2, 296], f32, tag="u12")
                nc.gpsimd.tensor_sub(u12[:, 1, :], c12v[:, 0, :], c12v[:, 1, :])
                nc.gpsimd.tensor_add(s2[:, 1, 2:298], s2[:, 1, 2:298], u12[:, 1, :])
                nc.vector.tensor_add(u12[:, 0, :], c12v[:, 0, :], c12v[:, 1, :])
                nc.vector.tensor_add(s2[:, 0, 2:298], s2[:, 0, 2:298], u12[:, 0, :])
            nc.sync.dma_start(rec_d[:], rec_buf[:])
    return nc


class _Results:
    """Minimal stand-in for BassKernelResults (no NTFF tracing under axon)."""

    def __init__(self, results):
        self.results = results
        self.instructions_and_trace = None
        self.profile_json = None
        self.exec_time_ns = None
        self.mean_exec_time_ns = None


def _warm_tunnel():
    """First bulk transfer on a cold axon tunnel runs at ~0.4 MB/s; a tiny
    transfer per device flips it to the fast path (~60 MB/s). Done in
    parallel: per-device cold-start after idle can cost ~15s each."""
    import jax
    from concurrent.futures import ThreadPoolExecutor

    def _touch(d):
        jax.device_put(np.zeros((4,), np.float32), d).block_until_ready()

    devs = jax.devices()[:8]
    with ThreadPoolExecutor(len(devs)) as ex:
        list(ex.map(_touch, devs))


def _make_runner(nc, n_cores=8):
    """Build a reusable jitted runner for the finalized Bass program.

    Mirrors bass2jax.run_bass_via_pjrt's multi-core branch, but hoists jit
    construction out of the per-call path so repeat calls skip recompilation
    (run_bass_kernel_spmd rebuilds the jit — and recompiles the NEFF — on
    every invocation)."""
    import jax
    from concourse import bass2jax
    from concourse import mybir

    bass2jax.install_neuronx_cc_hook()

    dbg_name = None
    if nc.dbg_addr is not None:
        if nc.dbg_callbacks:
            raise RuntimeError("dbg_callbacks unsupported under axon")
        dbg_name = nc.dbg_addr.name

    partition_name = nc.partition_id_tensor.name if nc.partition_id_tensor else None

    in_names, out_names, out_avals, zero_outs = [], [], [], []
    for alloc in nc.m.functions[0].allocations:
        if not isinstance(alloc, mybir.MemoryLocationSet):
            continue
        name = alloc.memorylocations[0].name
        if alloc.kind == "ExternalInput":
            if name != partition_name:
                in_names.append(name)
        elif alloc.kind == "ExternalOutput":
            shape = tuple(alloc.tensor_shape)
            dtype = mybir.dt.np(alloc.dtype)
            out_names.append(name)
            out_avals.append(jax.core.ShapedArray(shape, dtype))
            zero_outs.append(np.zeros(shape, dtype))
    n_params = len(in_names)
    n_outs = len(out_avals)
    in_names = in_names + out_names
    if partition_name is not None:
        in_names.append(partition_name)
    donate = tuple(range(n_params, n_params + n_outs))

    def _body(*args):
        operands = list(args)
        if partition_name is not None:
            operands.append(bass2jax.partition_id_tensor())
        outs = bass2jax._bass_exec_p.bind(
            *operands,
            out_avals=tuple(out_avals),
            in_names=tuple(in_names),
            out_names=tuple(out_names),
            lowering_input_output_aliases=(),
            sim_require_finite=True,
            sim_require_nnan=True,
            nc=nc,
        )
        return tuple(outs)

    devices = jax.devices()[:n_cores]
    mesh = bass2jax.Mesh(np.asarray(devices), ("core",))
    P_ = bass2jax.PartitionSpec("core")
    sharded = jax.jit(
        bass2jax.shard_map(
            _body, mesh=mesh, in_specs=(P_,) * (n_params + n_outs),
            out_specs=(P_,) * n_outs, check_rep=False
        ),
        donate_argnums=donate,
        keep_unused=True,
    )
    param_names = in_names[:n_params]

    # the constant weight slots never change: commit them to the devices once
    # with the sharding the jit expects, so calls skip that upload entirely
    wts_global = np.concatenate([_wts_np()] * n_cores, axis=0)
    wts_dev = jax.device_put(
        wts_global, jax.sharding.NamedSharding(mesh, P_))
    wts_dev.block_until_ready()

    def run(in_maps):
        if dbg_name is not None:
            in_maps = [{**m, dbg_name: np.zeros((1, 2), np.uint32)}
                       for m in in_maps]
        concat_in = [
            wts_dev if nm == "wts" else
            np.concatenate([np.asarray(in_maps[c][nm]) for c in range(n_cores)],
                           axis=0)
            for nm in param_names
        ]
        concat_zeros = [
            np.zeros((n_cores * z.shape[0], *z.shape[1:]), z.dtype)
            for z in zero_outs
        ]
        out_arrs = sharded(*concat_in, *concat_zeros)
        return [
            {nm: np.asarray(out_arrs[i]).reshape(n_cores, *out_avals[i].shape)[c]
             for i, nm in enumerate(out_names)}
            for c in range(n_cores)
        ]

    return run


def _zero_maps():
    return [{"cst": np.zeros((P, CTOT), np.float32),
             "srcamp": np.zeros((NSRC, NT), np.float32),
             "srcrow": np.zeros((NSRC, P), np.float32),
             "srcr": np.zeros((NSRC, W), np.float32),
             "rcol": np.zeros((NREC, W), np.float32)}
            for _ in range(8)]


def _ensure_ready():
    if "run" not in _cache:
        import os
        import sys
        import threading
        import time

        dbg = os.environ.get("BASS_KERNEL_TIMING") == "1"
        t0 = time.time()

        def mark(msg):
            if dbg:
                print(f"[kernel init {time.time() - t0:7.2f}s] {msg}",
                      file=sys.stderr, flush=True)

        warm = threading.Thread(target=_warm_tunnel)  # overlap with build
        warm.start()
        nc = build_nc(NT)
        mark("build_nc")
        nc.finalize()
        mark("finalize")
        _cache["nc"] = nc
        warm.join()
        mark("warm_tunnel joined")
        run = _make_runner(nc)
        mark("make_runner (incl wts upload)")
        # trigger trace + XLA + BIR->NEFF compile and warm the exec path
        try:
            run(_zero_maps())
        except Exception:
            if dbg:
                import traceback
                traceback.print_exc()
            raise
        mark("zeros run (trace+compile+exec)")
        _cache["run"] = run
    return _cache["run"]


def _run_fallback(in_maps):
    """Slow-but-robust path via the stock SPMD runner (recompiles per call)."""
    from concourse.bass_utils import run_bass_kernel_spmd
    nc = _cache.get("nc")
    if nc is None:
        nc = build_nc(NT)
        nc.finalize()
        _cache["nc"] = nc
    wts = _wts_np()
    in_maps = [{**m, "wts": wts} for m in in_maps]
    res = run_bass_kernel_spmd(nc, in_maps, core_ids=list(range(8)))
    return res.results


def kernel(lamb, mu, buoyancy, source_amplitudes_y,
           source_locations_y, receiver_locations_y, trace=False):
    amps = np.asarray(source_amplitudes_y, np.float32)
    src_loc = np.asarray(source_locations_y).astype(np.int64)
    rec_loc = np.asarray(receiver_locations_y).astype(np.int64)
    lambp, mup, buoyp, l2m, by, bx = _host_prep(
        np.asarray(lamb, np.float32), np.asarray(mu, np.float32),
        np.asarray(buoyancy, np.float32))

    in_maps = [
        _core_inputs(c, lambp, mup, buoyp, l2m, by, bx, amps, src_loc, rec_loc,
                     NT, 0)
        for c in range(8)
    ]
    try:
        run = _ensure_ready()
        res = run(in_maps)
    except Exception:
        res = _run_fallback(in_maps)
    kernel.last_results = _Results(res)

    out = np.zeros((N_SHOT, NREC, NT), np.float32)
    for s in range(N_SHOT):
        acc = res[4 * s]["rec"].astype(np.float32).copy()
        for j in range(1, 4):
            acc += res[4 * s + j]["rec"]
        out[s] = acc
    return out


import os as _os
if _os.environ.get("BASS_KERNEL_NO_INIT") != "1":
    try:
        _ensure_ready()
    except Exception:
        # fall back to lazy init inside kernel() (e.g. no devices at import)
        pass


# revision 33
# speedup vs baseline: 1.2275x; 1.2275x over previous
"""Elastic 2D velocity-stress FD (4th order, CPML) on 8 trn2 NeuronCores.

Sharding: 8 cores = 2 shots x 4 y-slabs (sizes [88,60,60,88]) with redundant
halos (each core owns a 128-row window of the 296-row padded grid; >=34-row
halos make the 64-step simulation exact to ~3e-9 with ZERO inter-core
communication — validated empirically against the reference).

Per-core layout: y on partitions (128), x on free dim (300 = 2 pad + 296 + 2 pad).
 - y-derivatives, CPML-y recursions, and all constant-coefficient linear
   combinations run on the TensorEngine as banded/diagonal matmuls accumulating
   into PSUM.
 - x-derivatives are 4 tap-matmuls (scaled identity x shifted-window rhs).
 - Only 2D-coefficient pointwise multiplies + CPML-x strip recursions run on
   VectorE; PSUM->SBUF copybacks on ScalarE.
 - Receivers are extracted on-chip each step (row-select matmul into PSUM +
   one DVE multiply-reduce against a column one-hot), so the only output is
   a [64, NT] trace block per core instead of the full wavefield.
Host does all per-core specialization (band matrices, coefficient fields,
source outer-product factors, receiver selectors); the final gather is a sum
of the 4 slab cores per shot.

The jit-wrapped executable is built and compiled once at import; the axon
tunnel is warmed with tiny transfers first (the first bulk transfer on a
cold tunnel runs ~150x slower than subsequent ones).
"""
import numpy as np

# --- problem constants (hardcoded per spec) ---
NY_I = NX_I = 256
PML = 20
DX = 4.0
DT = 5e-4
NT = 64
C1, C2 = 9.0 / 8.0, -1.0 / 24.0
NYP = NY_I + 2 * PML      # 296
NXP = NX_I + 2 * PML      # 296
W = NXP + 4               # 300 padded width; data cols 2..297
P = 128                   # partitions per core window
G0 = [0, 54, 114, 168]    # per-slab window start row (global padded coords)
SLABS = [(0, 88), (88, 148), (148, 208), (208, 296)]  # owned rows
NSRC = 8
NREC = 64
N_SHOT = 2
# x-stencil taps: d[x] = sum_k c_k * f[x+delta_k]
TAPC = [C1 / DX, -C1 / DX, C2 / DX, -C2 / DX]
DBWD = [0, -1, 1, -2]
DFWD = [1, 0, 2, -1]
# strip (x-PML) columns in padded coords: [2,22) and [278,298)
STRIP0 = [2, 278]
SW = 20

# packed const layout. The two bulk coefficient planes travel as bf16
# ("csth": ab2 pair + dt*buoy; note DT*(l2m-lamb)/2 == DT*mu, so ab2's B
# slot doubles as dtmu) and are widened to f32 on-chip; the small exact
# pieces stay f32 in "cst".
H_AB = 0                       # A/B coeff pair (2 x 300); B == dt*mu
H_DTB = H_AB + 2 * W           # dt*buoy        (300)
HTOT = H_DTB + W
C_BXS = 0                      # x-strip coeffs (2 sides x 20)
C_BY = C_BXS + 2 * SW
C_AY = C_BY + 1
C_RROW = C_AY + 1              # receiver row selector transposed (128 x 64)
CTOT = C_RROW + NREC

_cache = {}


def _host_prep(lamb, mu, buoyancy):
    f32 = np.float32
    lambp = np.pad(lamb.astype(f32), PML, mode='edge')
    mup = np.pad(mu.astype(f32), PML, mode='edge')
    buoyp = np.pad(buoyancy.astype(f32), PML, mode='edge')
    l2m = lambp + 2.0 * mup
    max_vel = np.max(np.sqrt(l2m * buoyp)).astype(f32)
    sig_max = f32(3.0 * max_vel * np.log(f32(1000.0)) / (2.0 * PML * DX))

    def prof(n):
        i = np.arange(n, dtype=f32)
        d = np.maximum(np.clip(PML - i, 0.0, None),
                       np.clip(i - (n - 1 - PML), 0.0, None)) / PML
        return sig_max * d * d

    by = np.exp(-prof(NYP) * f32(DT)).astype(f32)   # [296]
    bx = np.exp(-prof(NXP) * f32(DT)).astype(f32)   # [296]
    return lambp, mup, buoyp, l2m, by, bx


def _band(fwd):
    """Local [128,128] band matrix M with out = M @ f (rows=local out row)."""
    B = np.zeros((P, P), np.float32)
    taps = zip(DFWD if fwd else DBWD, TAPC)
    for off, c in taps:
        for m in range(P):
            k = m + off
            if 0 <= k < P:
                B[m, k] += c
    return B


def _core_inputs(core, lambp, mup, buoyp, l2m, by, bx, amps, src_loc, rec_loc,
                 nsteps, t0):
    """Build the ExternalInput dict for one core."""
    f32 = np.float32
    s, j = divmod(core, 4)
    g0 = G0[j]
    rs = slice(g0, g0 + P)
    byl = by[rs]
    ayl = byl - 1.0

    def widen(a):  # [128,296] -> [128,300] with zero pads
        out = np.zeros((P, W), f32)
        out[:, 2:2 + NXP] = a
        return out

    dtbuoy = widen(f32(DT) * buoyp[rs])
    A = widen(f32(DT) * (l2m[rs] + lambp[rs]) * 0.5)
    Bc = widen(f32(DT) * mup[rs])   # == DT*(l2m-lamb)/2 == dt*mu
    ab2 = np.stack([A, Bc], 1)      # [128,2,300]
    bxs = np.zeros((P, 2, SW), f32)
    for side, c0 in enumerate(STRIP0):
        bxs[:, side, :] = bx[c0 - 2:c0 - 2 + SW][None, :]

    srcamp = np.ascontiguousarray(amps[s, :, t0:t0 + nsteps]).astype(f32)
    srcrow = np.zeros((NSRC, P), f32)
    srcr = np.zeros((NSRC, W), f32)
    for i in range(NSRC):
        y = int(src_loc[s, i, 0]) + PML
        x = int(src_loc[s, i, 1]) + PML
        srcr[i, 2 + x] = 1.0
        if g0 <= y < g0 + P:
            srcrow[i, y - g0] = 1.0

    lo, hi = SLABS[j]
    rrowT = np.zeros((P, NREC), f32)
    rcol = np.zeros((NREC, W), f32)
    for r in range(NREC):
        y = int(rec_loc[s, r, 0]) + PML
        x = int(rec_loc[s, r, 1]) + PML
        if lo <= y < hi:
            rrowT[y - g0, r] = 1.0
            rcol[r, 2 + x] = 1.0

    import ml_dtypes
    csth = np.zeros((P, HTOT), np.float32)
    csth[:, H_AB:H_AB + 2 * W] = ab2.reshape(P, 2 * W)
    csth[:, H_DTB:H_DTB + W] = dtbuoy
    cst = np.zeros((P, CTOT), f32)
    cst[:, C_BXS:C_BXS + 2 * SW] = bxs.reshape(P, 2 * SW)
    cst[:, C_BY] = byl
    cst[:, C_AY] = ayl
    cst[:, C_RROW:C_RROW + NREC] = rrowT
    return {"cst": cst, "csth": csth.astype(ml_dtypes.bfloat16),
            "srcamp": srcamp, "srcrow": srcrow,
            "srcr": srcr, "rcol": rcol}


def _wts_np():
    """Constant band/tap weight slots [128, 6*128] (input-independent)."""
    f32 = np.float32
    eye = np.eye(P, dtype=f32)
    wts = np.zeros((P, 6, P), f32)
    wts[:, 0] = _band(fwd=False).T
    wts[:, 1] = _band(fwd=True).T
    for k in range(4):
        wts[:, 2 + k] = TAPC[k] * eye
    return wts.reshape(P, 6 * P)


def build_nc(nsteps=NT):
    import concourse.bacc as bacc
    import concourse.tile as tile
    from concourse import mybir

    f32 = mybir.dt.float32

    bf16 = mybir.dt.bfloat16
    nc = bacc.Bacc("TRN2", target_bir_lowering=False, debug=False, num_devices=8)
    cst_d = nc.dram_tensor("cst", [P, CTOT], f32, kind="ExternalInput")
    csth_d = nc.dram_tensor("csth", [P, HTOT], bf16, kind="ExternalInput")
    wts_d = nc.dram_tensor("wts", [P, 6 * P], f32, kind="ExternalInput")
    srcamp_d = nc.dram_tensor("srcamp", [NSRC, nsteps], f32, kind="ExternalInput")
    srcrow_d = nc.dram_tensor("srcrow", [NSRC, P], f32, kind="ExternalInput")
    srcr_d = nc.dram_tensor("srcr", [NSRC, W], f32, kind="ExternalInput")
    rcol_d = nc.dram_tensor("rcol", [NREC, W], f32, kind="ExternalInput")
    rec_d = nc.dram_tensor("rec", [NREC, nsteps], f32, kind="ExternalOutput")

    with tile.TileContext(nc) as tc:
        with (
            tc.tile_pool(name="const", bufs=1) as cp,
            tc.tile_pool(name="state", bufs=1) as sp,
            tc.tile_pool(name="scr", bufs=2) as scr,
            tc.tile_pool(name="ps", bufs=1, space="PSUM") as pp,
        ):
            cst = cp.tile([P, CTOT], f32)
            nc.sync.dma_start(cst[:], cst_d[:])
            csth = cp.tile([P, HTOT], bf16)
            nc.sync.dma_start(csth[:], csth_d[:])
            fields = cp.tile([P, HTOT], f32)
            nc.vector.tensor_copy(fields[:], csth[:])
            srcamp_t = cp.tile([NSRC, nsteps], f32)
            nc.sync.dma_start(srcamp_t[:], srcamp_d[:])
            srcrow_t = cp.tile([NSRC, P], f32)
            nc.sync.dma_start(srcrow_t[:], srcrow_d[:])
            srcr_t = cp.tile([NSRC, W], f32)
            nc.sync.dma_start(srcr_t[:], srcr_d[:])
            srcr = srcr_t[:]
            rcol_t = cp.tile([P, W], f32)
            nc.sync.dma_start(rcol_t[0:NREC, :], rcol_d[:])
            rcol = rcol_t[0:NREC, :]
            wtsr = cp.tile([P, 6 * P], f32)
            nc.sync.dma_start(wtsr[:], wts_d[:])
            # weights must be DVE-written so matmuls carry a single wait
            wts = cp.tile([P, 6, P], f32)
            nc.vector.tensor_copy(
                wts[:], wtsr[:].rearrange("p (a b) -> p a b", a=6))
            ab2 = fields[:, H_AB:H_AB + 2 * W].rearrange("p (a b) -> p a b", a=2)
            dtbuoy = fields[:, H_DTB:H_DTB + W]
            dtmu = ab2[:, 1, :]
            bxs = cst[:, C_BXS:C_BXS + 2 * SW].rearrange("p (a b) -> p a b", a=2)
            by_ap = cst[:, C_BY:C_BY + 1]
            ay_ap = cst[:, C_AY:C_AY + 1]
            rrowT = cst[:, C_RROW:C_RROW + NREC]

            v2 = sp.tile([P, 2, W], f32)      # vy | vx
            s2 = sp.tile([P, 2, W], f32)      # syy | sxx
            sxy = sp.tile([P, W], f32)
            my_vel = sp.tile([P, 2, W], f32)  # msyyy | msxyy
            my_str = sp.tile([P, 2, W], f32)  # mvyy | mvxy
            mw_vel = sp.tile([P, 2, W], f32)  # msxyx | msxxx (zero outside strips)
            mw_str = sp.tile([P, 2, W], f32)  # mvxx | mvyx
            rec_full = sp.tile([P, nsteps], f32)
            rec_buf = rec_full[0:NREC, :]
            for t_ in (v2, s2, sxy, my_vel, my_str, mw_vel, mw_str, rec_full):
                nc.vector.memset(t_[:], 0.0)

            ps_ab = pp.tile([P, 2, 512], f32)   # x-stencil taps: d_x pair
            ps_dy = pp.tile([P, 2, 512], f32)   # plain y-band derivs pair (+src)
            ps_st = pp.tile([P, 2, 512], f32)   # stress x-stencil taps pair
            ps_rec_full = pp.tile([P, 512], f32)  # receiver row-gather
            ps_rec = ps_rec_full[0:NREC, :]

            MM = nc.tensor.matmul
            Wt = lambda i: wts[:, i, :]
            vy, vx = v2[:, 0, :], v2[:, 1, :]

            def strips4v(ap2):
                """[P,20] per-var view at left strip -> [P,2,20] both strips."""
                a = ap2.copy()
                a.ap.insert(1, [STRIP0[1] - STRIP0[0], 2])
                return a

            def strip_chain_v(mw, f_, ps_pair):
                """Per-var CPML-x strip recursion (3 DVE ops, FD=40)."""
                d_ = strips4v(ps_pair[:, f_, STRIP0[0]:STRIP0[0] + SW])
                mwv = strips4v(mw[:, f_, STRIP0[0]:STRIP0[0] + SW])
                s_ = scr.tile([P, 2, SW], f32, tag="strip_s")
                nc.vector.tensor_add(s_[:], mwv, d_)
                nc.vector.tensor_mul(s_[:], s_[:], bxs[:])
                nc.vector.tensor_sub(mwv, s_[:], d_)

            Copy = mybir.ActivationFunctionType.Copy
            # expand source injection weights on-chip: z[i,t,:] = amp[i,t]*rowhot[i,:]
            zsrc_t = cp.tile([NSRC, nsteps, P], f32)
            for tt in range(nsteps):
                nc.scalar.activation(zsrc_t[:, tt, :], srcrow_t[:], Copy,
                                     scale=srcamp_t[:, tt:tt + 1])
            srcw = zsrc_t[:]
            for t in range(nsteps):
                sgc = dict(skip_group_check=True)
                # ================= VELOCITY =================
                # PE order: vy's inputs first (B@syy + src), so the vy chain
                # starts while PE still runs sxx taps.
                MM(ps_dy[:, 0, 2:298], Wt(0), s2[:, 0, 2:298], start=True, stop=False, **sgc)
                MM(ps_dy[:, 0, 2:298], srcw[:, t, :], srcr[:, 2:298],
                   start=False, stop=True, **sgc)
                for k in range(4):
                    d = DBWD[k]
                    MM(ps_ab[:, 0, 2:298], Wt(2 + k), sxy[:, 2 + d:298 + d],
                       start=(k == 0), stop=(k == 3), **sgc)
                MM(ps_dy[:, 1, 2:298], Wt(0), sxy[:, 2:298], start=True, stop=True, **sgc)
                # sxx x-derivative on DVE (PE tap block shrinks by 4 MMs):
                # tx = C1'*(f[x]-f[x-1]) + C2'*(f[x+1]-f[x-2]), real units
                tx = scr.tile([P, 296], f32, tag="tx")
                tt1 = scr.tile([P, 296], f32, tag="tt1")
                nc.vector.tensor_sub(tt1[:], s2[:, 1, 2:298], s2[:, 1, 1:297])
                nc.vector.tensor_sub(tx[:], s2[:, 1, 3:299], s2[:, 1, 0:296])
                nc.vector.scalar_tensor_tensor(
                    tx[:], tx[:], C2 / C1, tt1[:],
                    op0=mybir.AluOpType.mult, op1=mybir.AluOpType.add)
                nc.vector.tensor_scalar_mul(tx[:], tx[:], TAPC[0])
                # --- vy chain (DVE, reads PSUM directly) ---
                uy = scr.tile([P, 2, 296], f32, tag="uy")
                g0 = scr.tile([P, 296], f32, tag="g0")
                nc.scalar.activation(g0[:], my_vel[:, 0, 2:298], Copy, scale=by_ap)
                nc.scalar.activation(uy[:, 0, :], ps_dy[:, 0, 2:298], Copy, scale=ay_ap)
                nc.gpsimd.tensor_add(my_vel[:, 0, 2:298], g0[:], uy[:, 0, :])
                strip_chain_v(mw_vel, 0, ps_ab)
                # tree-parallel assembly: a1 = d_y+m' (DVE) || a2 = d_x+mw (ACT+Pool)
                S = scr.tile([P, 2, 296], f32, tag="S")
                wv = scr.tile([P, 2, 296], f32, tag="wv")
                e_ab0 = scr.tile([P, 296], f32, tag="e_ab0")
                a2 = scr.tile([P, 296], f32, tag="a2")
                nc.scalar.copy(e_ab0[:], ps_ab[:, 0, 2:298])
                nc.gpsimd.tensor_add(a2[:], e_ab0[:], mw_vel[:, 0, 2:298])
                nc.vector.tensor_add(S[:, 0, :], ps_dy[:, 0, 2:298], my_vel[:, 0, 2:298])
                nc.vector.tensor_add(S[:, 0, :], S[:, 0, :], a2[:])
                nc.vector.tensor_mul(wv[:, 0, :], dtbuoy[:, 2:298], S[:, 0, :])
                nc.vector.tensor_add(v2[:, 0, 2:298], v2[:, 0, 2:298], wv[:, 0, :])
                # --- receiver extraction: rec[r,t] = sum_xy Rrow[r,y] vy[y,x] Rcol[r,x]
                # (tensor_tensor_reduce wedges the TRN2 exec unit here; use
                #  separate multiply + reduce instead)
                MM(ps_rec[:, 0:W], rrowT, vy, start=True, stop=True, **sgc)
                rprod = scr.tile([P, W], f32, tag="rprod")
                nc.vector.tensor_mul(rprod[0:NREC, :], ps_rec[:, 0:W], rcol)
                nc.vector.reduce_sum(rec_buf[:, t:t + 1], rprod[0:NREC, :],
                                     axis=mybir.AxisListType.X)
                # --- vx chain (ACT drains PSUM, Pool arithmetic) ---
                nc.scalar.activation(uy[:, 1, :], ps_dy[:, 1, 2:298], Copy, scale=ay_ap)
                nc.vector.scalar_tensor_tensor(
                    my_vel[:, 1, 2:298], my_vel[:, 1, 2:298], by_ap, uy[:, 1, :],
                    op0=mybir.AluOpType.mult, op1=mybir.AluOpType.add)
                # var1 strip recursion off the SBUF-resident tx
                d1_ = strips4v(tx[:, 0:SW])
                mwv1 = strips4v(mw_vel[:, 1, STRIP0[0]:STRIP0[0] + SW])
                s1_ = scr.tile([P, 2, SW], f32, tag="strip_s")
                nc.vector.tensor_add(s1_[:], mwv1, d1_)
                nc.vector.tensor_mul(s1_[:], s1_[:], bxs[:])
                nc.vector.tensor_sub(mwv1, s1_[:], d1_)
                e_dy = scr.tile([P, 296], f32, tag="e_dy")
                nc.scalar.copy(e_dy[:], ps_dy[:, 1, 2:298])
                nc.gpsimd.tensor_add(S[:, 1, :], e_dy[:], my_vel[:, 1, 2:298])
                nc.gpsimd.tensor_add(S[:, 1, :], tx[:], S[:, 1, :])
                nc.gpsimd.tensor_add(S[:, 1, 0:296], S[:, 1, 0:296], mw_vel[:, 1, 2:298])
                nc.gpsimd.tensor_mul(wv[:, 1, :], dtbuoy[:, 2:298], S[:, 1, :])
                nc.gpsimd.tensor_add(v2[:, 1, 2:298], v2[:, 1, 2:298], wv[:, 1, :])

                # ================= STRESS =================
                # PE order: vy consumers first (vy finished first).
                MM(ps_dy[:, 0, 2:298], Wt(1), vy[:, 2:298], start=True, stop=True, **sgc)
                for k in range(4):
                    d = DFWD[k]
                    MM(ps_st[:, 1, 2:298], Wt(2 + k), vy[:, 2 + d:298 + d],
                       start=(k == 0), stop=(k == 3), **sgc)
                MM(ps_dy[:, 1, 2:298], Wt(1), vx[:, 2:298], start=True, stop=True, **sgc)
                for k in range(4):
                    d = DFWD[k]
                    MM(ps_st[:, 0, 2:298], Wt(2 + k), vx[:, 2 + d:298 + d],
                       start=(k == 0), stop=(k == 3), **sgc)
                uy2 = scr.tile([P, 2, 296], f32, tag="uy")
                # --- sxy chain (finish first: next velocity needs sxy) ---
                g1 = scr.tile([P, 296], f32, tag="g0")
                nc.scalar.activation(g1[:], my_str[:, 1, 2:298], Copy, scale=by_ap)
                nc.scalar.activation(uy2[:, 1, :], ps_dy[:, 1, 2:298], Copy, scale=ay_ap)
                nc.gpsimd.tensor_add(my_str[:, 1, 2:298], g1[:], uy2[:, 1, :])
                strip_chain_v(mw_str, 1, ps_st)
                T2 = scr.tile([P, 2, 296], f32, tag="T2")
                X2 = scr.tile([P, 2, 296], f32, tag="X2")
                e_t = scr.tile([P, 296], f32, tag="e_t")
                nc.scalar.copy(e_t[:], ps_dy[:, 1, 2:298])
                nc.gpsimd.tensor_add(T2[:, 1, :], e_t[:], my_str[:, 1, 2:298])
                nc.vector.tensor_add(X2[:, 1, :], ps_st[:, 1, 2:298], mw_str[:, 1, 2:298])
                t5 = scr.tile([P, 296], f32, tag="t5")
                nc.gpsimd.tensor_add(t5[:], T2[:, 1, :], X2[:, 1, :])
                nc.gpsimd.tensor_mul(t5[:], dtmu[:, 2:298], t5[:])
                nc.gpsimd.tensor_add(sxy[:, 2:298], sxy[:, 2:298], t5[:])
                # --- syy/sxx chain; sxx finishes before syy (taps need sxx) ---
                nc.scalar.activation(uy2[:, 0, :], ps_dy[:, 0, 2:298], Copy, scale=ay_ap)
                nc.vector.scalar_tensor_tensor(
                    my_str[:, 0, 2:298], my_str[:, 0, 2:298], by_ap, uy2[:, 0, :],
                    op0=mybir.AluOpType.mult, op1=mybir.AluOpType.add)
                strip_chain_v(mw_str, 0, ps_st)
                nc.vector.tensor_add(T2[:, 0, :], ps_dy[:, 0, 2:298], my_str[:, 0, 2:298])
                nc.vector.tensor_add(X2[:, 0, :], ps_st[:, 0, 2:298], mw_str[:, 0, 2:298])
                tpm = scr.tile([P, 2, 296], f32, tag="tpm")
                nc.vector.tensor_add(tpm[:, 0, :], T2[:, 0, :], X2[:, 0, :])
                nc.gpsimd.tensor_sub(tpm[:, 1, :], T2[:, 0, :], X2[:, 0, :])
                c12v = scr.tile([P, 2, 296], f32, tag="c12v")
                nc.vector.tensor_mul(c12v[:], ab2[:, :, 2:298], tpm[:])
                u12 = scr.tile([P, 2, 296], f32, tag="u12")
                nc.gpsimd.tensor_sub(u12[:, 1, :], c12v[:, 0, :], c12v[:, 1, :])
                nc.gpsimd.tensor_add(s2[:, 1, 2:298], s2[:, 1, 2:298], u12[:, 1, :])
                nc.vector.tensor_add(u12[:, 0, :], c12v[:, 0, :], c12v[:, 1, :])
                nc.vector.tensor_add(s2[:, 0, 2:298], s2[:, 0, 2:298], u12[:, 0, :])
            nc.sync.dma_start(rec_d[:], rec_buf[:])
    return nc


class _Results:
    """Minimal stand-in for BassKernelResults (no NTFF tracing under axon)."""

    def __init__(self, results):
        self.results = results
        self.instructions_and_trace = None
        self.profile_json = None
        self.exec_time_ns = None
        self.mean_exec_time_ns = None


def _warm_tunnel():
    """First bulk transfer on a cold axon tunnel runs at ~0.4 MB/s; a tiny
    transfer per device flips it to the fast path (~60 MB/s). Done in
    parallel: per-device cold-start after idle can cost ~15s each."""
    import jax
    from concurrent.futures import ThreadPoolExecutor

    def _touch(d):
        jax.device_put(np.zeros((4,), np.float32), d).block_until_ready()

    devs = jax.devices()[:8]
    with ThreadPoolExecutor(len(devs)) as ex:
        list(ex.map(_touch, devs))


def _make_runner(nc, n_cores=8):
    """Build a reusable jitted runner for the finalized Bass program.

    Mirrors bass2jax.run_bass_via_pjrt's multi-core branch, but hoists jit
    construction out of the per-call path so repeat calls skip recompilation
    (run_bass_kernel_spmd rebuilds the jit — and recompiles the NEFF — on
    every invocation)."""
    import jax
    from concourse import bass2jax
    from concourse import mybir

    bass2jax.install_neuronx_cc_hook()

    dbg_name = None
    if nc.dbg_addr is not None:
        if nc.dbg_callbacks:
            raise RuntimeError("dbg_callbacks unsupported under axon")
        dbg_name = nc.dbg_addr.name

    partition_name = nc.partition_id_tensor.name if nc.partition_id_tensor else None

    in_names, out_names, out_avals, zero_outs = [], [], [], []
    for alloc in nc.m.functions[0].allocations:
        if not isinstance(alloc, mybir.MemoryLocationSet):
            continue
        name = alloc.memorylocations[0].name
        if alloc.kind == "ExternalInput":
            if name != partition_name:
                in_names.append(name)
        elif alloc.kind == "ExternalOutput":
            shape = tuple(alloc.tensor_shape)
            dtype = mybir.dt.np(alloc.dtype)
            out_names.append(name)
            out_avals.append(jax.core.ShapedArray(shape, dtype))
            zero_outs.append(np.zeros(shape, dtype))
    n_params = len(in_names)
    n_outs = len(out_avals)
    in_names = in_names + out_names
    if partition_name is not None:
        in_names.append(partition_name)
    donate = tuple(range(n_params, n_params + n_outs))

    def _body(*args):
        operands = list(args)
        if partition_name is not None:
            operands.append(bass2jax.partition_id_tensor())
        outs = bass2jax._bass_exec_p.bind(
            *operands,
            out_avals=tuple(out_avals),
            in_names=tuple(in_names),
            out_names=tuple(out_names),
            lowering_input_output_aliases=(),
            sim_require_finite=True,
            sim_require_nnan=True,
            nc=nc,
        )
        return tuple(outs)

    devices = jax.devices()[:n_cores]
    mesh = bass2jax.Mesh(np.asarray(devices), ("core",))
    P_ = bass2jax.PartitionSpec("core")
    sharded = jax.jit(
        bass2jax.shard_map(
            _body, mesh=mesh, in_specs=(P_,) * (n_params + n_outs),
            out_specs=(P_,) * n_outs, check_rep=False
        ),
        donate_argnums=donate,
        keep_unused=True,
    )
    param_names = in_names[:n_params]

    # the constant weight slots never change: commit them to the devices once
    # with the sharding the jit expects, so calls skip that upload entirely
    wts_global = np.concatenate([_wts_np()] * n_cores, axis=0)
    wts_dev = jax.device_put(
        wts_global, jax.sharding.NamedSharding(mesh, P_))
    wts_dev.block_until_ready()

    def run(in_maps):
        if dbg_name is not None:
            in_maps = [{**m, dbg_name: np.zeros((1, 2), np.uint32)}
                       for m in in_maps]
        concat_in = [
            wts_dev if nm == "wts" else
            np.concatenate([np.asarray(in_maps[c][nm]) for c in range(n_cores)],
                           axis=0)
            for nm in param_names
        ]
        concat_zeros = [
            np.zeros((n_cores * z.shape[0], *z.shape[1:]), z.dtype)
            for z in zero_outs
        ]
        out_arrs = sharded(*concat_in, *concat_zeros)
        return [
            {nm: np.asarray(out_arrs[i]).reshape(n_cores, *out_avals[i].shape)[c]
             for i, nm in enumerate(out_names)}
            for c in range(n_cores)
        ]

    return run


def _zero_maps():
    import ml_dtypes
    return [{"cst": np.zeros((P, CTOT), np.float32),
             "csth": np.zeros((P, HTOT), ml_dtypes.bfloat16),
             "srcamp": np.zeros((NSRC, NT), np.float32),
             "srcrow": np.zeros((NSRC, P), np.float32),
             "srcr": np.zeros((NSRC, W), np.float32),
             "rcol": np.zeros((NREC, W), np.float32)}
            for _ in range(8)]


def _ensure_ready():
    if "run" not in _cache:
        import os
        import sys
        import threading
        import time

        dbg = os.environ.get("BASS_KERNEL_TIMING") == "1"
        t0 = time.time()

        def mark(msg):
            if dbg:
                print(f"[kernel init {time.time() - t0:7.2f}s] {msg}",
                      file=sys.stderr, flush=True)

        warm = threading.Thread(target=_warm_tunnel)  # overlap with build
        warm.start()
        nc = build_nc(NT)
        mark("build_nc")
        nc.finalize()
        mark("finalize")
        _cache["nc"] = nc
        warm.join()
        mark("warm_tunnel joined")
        run = _make_runner(nc)
        mark("make_runner (incl wts upload)")
        # trigger trace + XLA + BIR->NEFF compile and warm the exec path
        try:
            run(_zero_maps())
        except Exception:
            if dbg:
                import traceback
                traceback.print_exc()
            raise
        mark("zeros run (trace+compile+exec)")
        _cache["run"] = run
    return _cache["run"]


def _run_fallback(in_maps):
    """Slow-but-robust path via the stock SPMD runner (recompiles per call)."""
    from concourse.bass_utils import run_bass_kernel_spmd
    nc = _cache.get("nc")
    if nc is None:
        nc = build_nc(NT)
        nc.finalize()
        _cache["nc"] = nc
    wts = _wts_np()
    in_maps = [{**m, "wts": wts} for m in in_maps]
    res = run_bass_kernel_spmd(nc, in_maps, core_ids=list(range(8)))
    return res.results


def kernel(lamb, mu, buoyancy, source_amplitudes_y,
           source_locations_y, receiver_locations_y, trace=False):
    amps = np.asarray(source_amplitudes_y, np.float32)
    src_loc = np.asarray(source_locations_y).astype(np.int64)
    rec_loc = np.asarray(receiver_locations_y).astype(np.int64)
    lambp, mup, buoyp, l2m, by, bx = _host_prep(
        np.asarray(lamb, np.float32), np.asarray(mu, np.float32),
        np.asarray(buoyancy, np.float32))

    in_maps = [
        _core_inputs(c, lambp, mup, buoyp, l2m, by, bx, amps, src_loc, rec_loc,
                     NT, 0)
        for c in range(8)
    ]
    try:
        run = _ensure_ready()
        res = run(in_maps)
    except Exception:
        res = _run_fallback(in_maps)
    kernel.last_results = _Results(res)

    out = np.zeros((N_SHOT, NREC, NT), np.float32)
    for s in range(N_SHOT):
        acc = res[4 * s]["rec"].astype(np.float32).copy()
        for j in range(1, 4):
            acc += res[4 * s + j]["rec"]
        out[s] = acc
    return out


import os as _os
if _os.environ.get("BASS_KERNEL_NO_INIT") != "1":
    try:
        _ensure_ready()
    except Exception:
        # fall back to lazy init inside kernel() (e.g. no devices at import)
        pass
